# revision 1
# baseline (speedup 1.0000x reference)
"""DiT flow model forward pass on 8 Trainium2 NeuronCores.

Data-parallel over batch (8 batches/core, T=256 tokens/core), weights
replicated. Activations are kept transposed [D, T] on-chip so the whole layer
chain runs without activation transposes. Weights are pre-transposed to
K-major on the host.

Precision scheme: matmul operands (weights + matmul-facing activation copies)
are W_DT (default bf16: 2x PE rate, half DMA traffic); the residual stream,
layernorm statistics and all PSUM accumulation stay fp32 (fp32r-tagged tiles
stream at full PE rate for the fp32 matmuls that remain).

The hidden dimension is permuted per-head (even rotary slots first, odd
second) so RoPE becomes elementwise muls plus a contiguous 32-partition block
swap; the permutation is folded into the weights on the host.
"""

import sys

sys.path.insert(0, "/opt/trn_rl_repo")

from contextlib import ExitStack

import ml_dtypes
import numpy as np

import bass_rust
import concourse.bass as bass
import concourse.mybir as mybir
import concourse.tile as tile
from concourse.bass_utils import run_bass_kernel_spmd
from concourse.vector_clock import ScopedClock

B, S, LD, Hh, Ww = 64, 32, 16, 32, 18
D, NH, HD, FF, L = 512, 8, 64, 2048, 6
IN = LD * Hh * Ww
EPS = 1e-5
NCORES = 8
BSH = B // NCORES          # 8 batches per core
T = BSH * S                # 256 tokens per core
NEG = -30000.0             # additive mask value; bf16-exact, exp() underflows to 0

f32 = mybir.dt.float32
f32r = mybir.dt.float32r
bf16 = mybir.dt.bfloat16
AT = mybir.ActivationFunctionType
ALU = mybir.AluOpType

# matmul-operand dtype for layer weights / attention / ffn, and for the big
# input/output projections.
W_DT = bf16
IO_DT = bf16

# ---------------------------------------------------------------------------
# walrus in this container accepts at most ONE inline sync-wait per
# instruction; Tile can attach several. Split extras onto NoOp carriers.
# ---------------------------------------------------------------------------

def _patched_drain_and_barrier(self, tick_clock, wait_clock):
    nc = self.nc
    ticks = list(tick_clock.global_clock)
    for p, t in enumerate(ticks):
        if t > 0:
            vc = bass_rust.VectorClock([t if i == p else 0 for i in range(len(ticks))])
            nop_inst = nc.sync.nop(nofuse=True, hint=f"tailw{p}")
            wait_clock.add_sem_waits(nop_inst.ins, ScopedClock({None: vc}))
    nc.sync.drain()
    nc.all_engine_barrier()
    popped = nc._tile_sem_poison_stack.pop()
    assert popped is self._sem_poison
    nc.clear_and_free_semaphores(list(self.sems.allocated().values()))
    nc.all_engine_barrier()


def _split_multi_waits(nc, max_waits=1):
    for f in nc.m.functions:
        for blk in f.blocks:
            idx = 0
            while idx < len(blk.instructions):
                inst = blk.instructions[idx]
                si = inst.sync_info
                if si is not None and len(si.on_wait) > max_waits:
                    waits = list(si.on_wait)
                    for j, w in enumerate(waits[:-max_waits]):
                        carrier = mybir.InstNoOp(
                            name=f"{inst.name}_wsplit{j}",
                            engine=inst.engine,
                            bass_nofuse=True,
                            sync_info=mybir.SyncInfo(on_wait=[w], on_update=[]),
                        )
                        nc.register_instruction(carrier)
                        blk.instructions.insert(idx, carrier)
                        idx += 1
                    si.on_wait = waits[-max_waits:]
                idx += 1


tile.TileContext._drain_and_barrier = _patched_drain_and_barrier

# ---------------------------------------------------------------------------
# host-side numerics helpers
# ---------------------------------------------------------------------------

def _round_f32r(x):
    b = np.ascontiguousarray(x, dtype=np.float32).view(np.uint32)
    b = (b + np.uint32(0x7FF) + ((b >> np.uint32(12)) & np.uint32(1))) & np.uint32(0xFFFFF000)
    return b.view(np.float32)


def _cast(x, dt):
    if dt is bf16:
        return np.ascontiguousarray(np.asarray(x, np.float32)).astype(ml_dtypes.bfloat16)
    return _round_f32r(np.ascontiguousarray(x))


def _perm_src():
    p = np.empty(D, dtype=np.int64)
    for h in range(NH):
        for j in range(HD // 2):
            p[h * HD + j] = h * HD + 2 * j
            p[h * HD + HD // 2 + j] = h * HD + 2 * j + 1
    return p


# ---------------------------------------------------------------------------
# Bass kernel build
# ---------------------------------------------------------------------------

_CACHE = {}


def _build(nlayers):
    nc = bass.Bass()

    xT = nc.dram_tensor("xT", [IN, T], IO_DT, kind="ExternalInput")
    inp_wT = nc.dram_tensor("inp_wT", [IN, D], IO_DT, kind="ExternalInput")
    outp_wT = nc.dram_tensor("outp_wT", [D, IN], IO_DT, kind="ExternalInput")
    w_qkv = [nc.dram_tensor(f"w_qkv_{l}", [D, 3 * D], W_DT, kind="ExternalInput") for l in range(nlayers)]
    w_out = [nc.dram_tensor(f"w_out_{l}", [D, D], W_DT, kind="ExternalInput") for l in range(nlayers)]
    w_ff1 = [nc.dram_tensor(f"w_ff1_{l}", [D, FF], W_DT, kind="ExternalInput") for l in range(nlayers)]
    w_ff2 = [nc.dram_tensor(f"w_ff2_{l}", [FF, D], W_DT, kind="ExternalInput") for l in range(nlayers)]
    ct_d = nc.dram_tensor("ct", [D, T], f32, kind="ExternalInput")
    sts_d = nc.dram_tensor("sts", [D, T], f32, kind="ExternalInput")
    maskT_d = nc.dram_tensor("maskT", [128, 128], W_DT, kind="ExternalInput")
    idw_d = nc.dram_tensor("idw", [128, 128], W_DT, kind="ExternalInput")
    onesr_d = nc.dram_tensor("onesr", [128, 128], f32r, kind="ExternalInput")
    onesw_d = nc.dram_tensor("onesw", [1, 128], W_DT, kind="ExternalInput")
    pswap_d = nc.dram_tensor("pswap", [128, 128], f32r, kind="ExternalInput")
    lnc_d = nc.dram_tensor("lnc", [D, 4 * nlayers], f32, kind="ExternalInput")
    qkb_d = nc.dram_tensor("qkb", [D, 2 * nlayers], f32, kind="ExternalInput")
    obt_d = nc.dram_tensor("obt", [D, nlayers], f32, kind="ExternalInput")
    ff1b_d = nc.dram_tensor("ff1bt", [FF, nlayers], f32, kind="ExternalInput")
    ff2b_d = nc.dram_tensor("ff2bt", [D, nlayers], f32, kind="ExternalInput")
    inpb_d = nc.dram_tensor("inpbt", [D, 1], f32, kind="ExternalInput")
    bvr_d = nc.dram_tensor("bvr", [nlayers, D], W_DT, kind="ExternalInput")
    out_d = nc.dram_tensor("out", [T, IN], f32, kind="ExternalOutput")

    with tile.TileContext(nc) as tc, ExitStack() as top:
        cp = top.enter_context(tc.tile_pool(name="consts", bufs=1))
        ap = top.enter_context(tc.tile_pool(name="acts", bufs=10))
        stp = top.enter_context(tc.tile_pool(name="stats", bufs=8))
        atp = top.enter_context(tc.tile_pool(name="attn", bufs=8))

        # ---- constants -----------------------------------------------------
        ct = cp.tile([128, 4, T], f32, tag="ct")
        nc.sync.dma_start(ct[:], ct_d.rearrange("(kt p) t -> p kt t", p=128))
        sts = cp.tile([128, 4, T], f32, tag="sts")
        nc.sync.dma_start(sts[:], sts_d.rearrange("(kt p) t -> p kt t", p=128))
        maskT = cp.tile([128, 128], W_DT, tag="maskT")
        nc.sync.dma_start(maskT[:], maskT_d[:])
        idw = cp.tile([128, 128], W_DT, tag="idw")
        nc.sync.dma_start(idw[:], idw_d[:])
        onesr = cp.tile([128, 128], f32r, tag="onesr")
        nc.sync.dma_start(onesr[:], onesr_d[:])
        onesw = cp.tile([1, 128], W_DT, tag="onesw")
        nc.sync.dma_start(onesw[:], onesw_d[:])
        pswap = cp.tile([128, 128], f32r, tag="pswap")
        nc.sync.dma_start(pswap[:], pswap_d[:])
        lnc = cp.tile([128, 4, 4 * nlayers], f32, tag="lnc")
        nc.sync.dma_start(lnc[:], lnc_d.rearrange("(kt p) n -> p kt n", p=128))
        qkb = cp.tile([128, 4, 2 * nlayers], f32, tag="qkb")
        nc.sync.dma_start(qkb[:], qkb_d.rearrange("(kt p) n -> p kt n", p=128))
        obt = cp.tile([128, 4, nlayers], f32, tag="obt")
        nc.sync.dma_start(obt[:], obt_d.rearrange("(kt p) n -> p kt n", p=128))
        ff1b = cp.tile([128, 16, nlayers], f32, tag="ff1b")
        nc.sync.dma_start(ff1b[:], ff1b_d.rearrange("(kt p) n -> p kt n", p=128))
        ff2b = cp.tile([128, 4, nlayers], f32, tag="ff2b")
        nc.sync.dma_start(ff2b[:], ff2b_d.rearrange("(kt p) n -> p kt n", p=128))
        inpb = cp.tile([128, 4, 1], f32, tag="inpb")
        nc.sync.dma_start(inpb[:], inpb_d.rearrange("(kt p) n -> p kt n", p=128))
        epsc = cp.tile([1, 1], f32, tag="epsc")
        nc.vector.memset(epsc[:], EPS)

        hT = ap.tile([128, 4, T], f32r, tag="act")

        # layer-phase pools opened first so layer-0 weights prefetch during
        # the input projection (stack allocator: inp pools nest inside)
        wp = top.enter_context(tc.tile_pool(name="wl", bufs=2))
        glp = top.enter_context(tc.tile_pool(name="gl", bufs=2))
        vp = top.enter_context(tc.tile_pool(name="vp", bufs=2))
        pmm = top.enter_context(tc.tile_pool(name="ps_mm", bufs=4, space="PSUM"))
        patt = top.enter_context(tc.tile_pool(name="ps_att", bufs=4, space="PSUM"))

        # ---- input projection: hT[D, T] = (x @ inp_w.T).T ------------------
        KT_IN = IN // 128          # 72 k-tiles
        CH = 9                     # k-tiles per streamed chunk
        with tc.tile_pool(name="inp_sb", bufs=2) as ip:
            hps = [pmm.tile([128, T], f32, tag="mm", name=f"hps{m}") for m in range(4)]
            for kc in range(KT_IN // CH):
                xc = ip.tile([128, CH, T], IO_DT, tag="xc")
                nc.sync.dma_start(
                    xc[:], xT[kc * CH * 128:(kc + 1) * CH * 128, :]
                    .rearrange("(kt p) t -> p kt t", p=128))
                wc = ip.tile([128, CH, D], IO_DT, tag="wc")
                nc.sync.dma_start(
                    wc[:], inp_wT[kc * CH * 128:(kc + 1) * CH * 128, :]
                    .rearrange("(kt p) n -> p kt n", p=128))
                for kk in range(CH):
                    first = kc == 0 and kk == 0
                    last = kc == KT_IN // CH - 1 and kk == CH - 1
                    for m in range(4):
                        nc.tensor.matmul(hps[m][:], wc[:, kk, m * 128:(m + 1) * 128],
                                         xc[:, kk, :], start=first, stop=last)
            for m in range(4):
                nc.scalar.activation(hT[:, m], hps[m][:], AT.Identity,
                                     bias=inpb[:, m, 0:1], scale=1.0)

        # ---- transformer layers -------------------------------------------
        if True:

            def layernorm(src, wb_idx, dst, dstb):
                """src f32r [128,4,T]; writes f32r tile dst and W_DT copy dstb."""
                sum_ps = pmm.tile([1, T], f32, tag="mm")
                sq = ap.tile([128, 4, T], f32r, tag="act")
                sq_ps = pmm.tile([1, T], f32, tag="mm")
                for k in range(4):
                    nc.tensor.matmul(sum_ps[:], onesr[:, 0:1], src[:, k],
                                     start=(k == 0), stop=(k == 3))
                    nc.scalar.activation(sq[:, k], src[:, k].bitcast(f32), AT.Square)
                    nc.tensor.matmul(sq_ps[:], onesr[:, 0:1], sq[:, k],
                                     start=(k == 0), stop=(k == 3))
                mu = stp.tile([1, T], f32, tag="st")
                nc.vector.tensor_scalar_mul(mu[:], sum_ps[:], 1.0 / D)
                ex2 = stp.tile([1, T], f32, tag="st")
                nc.vector.tensor_scalar_mul(ex2[:], sq_ps[:], 1.0 / D)
                mu2 = stp.tile([1, T], f32, tag="st")
                nc.vector.tensor_mul(mu2[:], mu[:], mu[:])
                var = stp.tile([1, T], f32, tag="st")
                nc.vector.tensor_tensor(var[:], ex2[:], mu2[:], ALU.subtract)
                sd = stp.tile([1, T], f32, tag="st")
                nc.scalar.activation(sd[:], var[:], AT.Sqrt, bias=epsc[0:1, 0:1], scale=1.0)
                rstd = stp.tile([1, T], f32r, tag="st")
                with nc.allow_low_precision(reason="rstd rounded to f32r for PE broadcast"):
                    nc.vector.reciprocal(rstd[:], sd[:])
                ms = stp.tile([1, T], f32r, tag="st")
                nc.vector.tensor_mul(ms[:], mu[:], rstd[:].bitcast(f32))
                rstdB = pmm.tile([128, T], f32, tag="mm")
                nc.tensor.matmul(rstdB[:], onesr[0:1, :], rstd[:], start=True, stop=True)
                msB = pmm.tile([128, T], f32, tag="mm")
                nc.tensor.matmul(msB[:], onesr[0:1, :], ms[:], start=True, stop=True)
                t0 = ap.tile([128, 4, T], f32, tag="lnsc", bufs=2)
                t1 = ap.tile([128, 4, T], f32, tag="lnsc", bufs=2)
                for m in range(4):
                    nc.vector.tensor_mul(t0[:, m], src[:, m].bitcast(f32), rstdB[:])
                    nc.vector.tensor_tensor(t1[:, m], t0[:, m], msB[:], ALU.subtract)
                    nc.scalar.activation(dst[:, m], t1[:, m], AT.Identity,
                                         bias=lnc[:, m, wb_idx + 1:wb_idx + 2],
                                         scale=lnc[:, m, wb_idx:wb_idx + 1])
                    nc.scalar.activation(dstb[:, m], dst[:, m].bitcast(f32), AT.Identity)

            # W_DT copy of hT for v-projection lhsT
            hTb = ap.tile([128, 4, T], W_DT, tag="actb")
            nc.scalar.activation(hTb[:], hT[:].bitcast(f32), AT.Identity)

            for l in range(nlayers):
                bvl = stp.tile([1, D], W_DT, tag="bv", bufs=2)
                nc.sync.dma_start(bvl[:], bvr_d[l:l + 1, :])
                wqkv = wp.tile([128, 4, 3 * D], W_DT, tag="w")
                nc.sync.dma_start(wqkv[:], w_qkv[l].rearrange("(kt p) n -> p kt n", p=128))
                wout = wp.tile([128, 4, D], W_DT, tag="w")
                nc.sync.dma_start(wout[:], w_out[l].rearrange("(kt p) n -> p kt n", p=128))
                wff1 = wp.tile([128, 4, FF], W_DT, tag="w")
                nc.sync.dma_start(wff1[:], w_ff1[l].rearrange("(kt p) n -> p kt n", p=128))
                wff2 = wp.tile([128, 16, D], W_DT, tag="w")
                nc.sync.dma_start(wff2[:], w_ff2[l].rearrange("(kt p) n -> p kt n", p=128))

                # RoPE on hT -> hrT (pairs are (j, j+32) blocks within each head)
                hrT = ap.tile([128, 4, T], W_DT, tag="actb")
                t2 = ap.tile([128, 4, T], f32, tag="ropesc", bufs=2)
                t1 = ap.tile([128, 4, T], f32, tag="ropesc", bufs=2)
                for m in range(4):
                    swp_ps = pmm.tile([128, T], f32, tag="mm")
                    nc.tensor.matmul(swp_ps[:], pswap[:], hT[:, m],
                                     start=True, stop=True)
                    nc.vector.tensor_mul(t2[:, m], hT[:, m].bitcast(f32), ct[:, m])
                    nc.vector.tensor_mul(t1[:, m], swp_ps[:], sts[:, m])
                    nc.vector.tensor_add(hrT[:, m], t2[:, m], t1[:, m])

                # q/k projections (Form T: out [Do,T])
                qT = ap.tile([128, 4, T], W_DT, tag="actb")
                kT = ap.tile([128, 4, T], W_DT, tag="actb")
                for qk, dst in ((0, qT), (1, kT)):
                    for m in range(4):
                        ps = pmm.tile([128, T], f32, tag="mm")
                        for k in range(4):
                            nc.tensor.matmul(
                                ps[:], wqkv[:, k, qk * D + m * 128: qk * D + (m + 1) * 128],
                                hrT[:, k], start=(k == 0), stop=(k == 3))
                        nc.scalar.activation(dst[:, m], ps[:], AT.Identity,
                                             bias=qkb[:, m, 2 * l + qk: 2 * l + qk + 1],
                                             scale=1.0)

                # v projection (Form N: out [T,D]) + bias via K=1 ones matmul
                v = vp.tile([128, 2, D], W_DT, tag="v")
                for m2 in range(2):
                    ps = pmm.tile([128, D], f32, tag="mm")
                    for k in range(4):
                        nc.tensor.matmul(ps[:], hTb[:, k, m2 * 128:(m2 + 1) * 128],
                                         wqkv[:, k, 2 * D:3 * D],
                                         start=(k == 0), stop=False)
                    nc.tensor.matmul(ps[:], onesw[:], bvl[:],
                                     start=False, stop=True)
                    nc.vector.tensor_copy(v[:, m2], ps[:])

                # attention per (head-pair kt, head, token-half)
                ctxT = ap.tile([128, 4, T], W_DT, tag="actb")
                for kt in range(4):
                    cps = pmm.tile([128, T], f32, tag="mm")
                    for hh in range(2):
                        h = 2 * kt + hh
                        pb = 64 * hh
                        attn = atp.tile([128, 256], W_DT, tag="atb")
                        for half in range(2):
                            fr = slice(half * 128, (half + 1) * 128)
                            sc = patt.tile([128, 128], f32, tag="sc")
                            nc.tensor.matmul(sc[:], maskT[:], idw[:], start=True, stop=False)
                            nc.tensor.matmul(sc[:], qT[pb:pb + 64, kt, fr],
                                             kT[pb:pb + 64, kt, fr], start=False, stop=True)
                            att = atp.tile([128, 128], f32, tag="at")
                            rsum = stp.tile([128, 1], f32, tag="rs")
                            nc.scalar.activation(att[:], sc[:], AT.Exp, accum_out=rsum[:])
                            rinv = stp.tile([128, 1], f32, tag="rs")
                            nc.vector.reciprocal(rinv[:], rsum[:])
                            nc.vector.tensor_scalar_mul(attn[:, half * 128:(half + 1) * 128],
                                                        att[:], rinv[:])
                        atT = atp.tile([128, 256], W_DT, tag="atb")
                        nc.vector.transpose(atT[:], attn[:])
                        for half in range(2):
                            fr = slice(half * 128, (half + 1) * 128)
                            nc.tensor.matmul(cps[pb:pb + 64, fr],
                                             v[:, half, h * 64:(h + 1) * 64],
                                             atT[:, fr], start=True, stop=True)
                    nc.vector.tensor_copy(ctxT[:, kt, :], cps[:])

                # out projection + residual
                h1pre = ap.tile([128, 4, T], f32r, tag="act")
                sa4 = ap.tile([128, 4, T], f32, tag="resc", bufs=2)
                for m in range(4):
                    ps = pmm.tile([128, T], f32, tag="mm")
                    for k in range(4):
                        nc.tensor.matmul(ps[:], wout[:, k, m * 128:(m + 1) * 128],
                                         ctxT[:, k], start=(k == 0), stop=(k == 3))
                    nc.scalar.activation(sa4[:, m], ps[:], AT.Identity,
                                         bias=obt[:, m, l:l + 1], scale=1.0)
                    nc.vector.tensor_add(h1pre[:, m], sa4[:, m], hT[:, m].bitcast(f32))

                h1T = ap.tile([128, 4, T], f32r, tag="act")
                h1Tb = ap.tile([128, 4, T], W_DT, tag="actb")
                layernorm(h1pre, 4 * l, h1T, h1Tb)

                # FFN
                gl = glp.tile([128, 16, T], W_DT, tag="gl")
                for ft in range(16):
                    ps = pmm.tile([128, T], f32, tag="mm")
                    for k in range(4):
                        nc.tensor.matmul(ps[:], wff1[:, k, ft * 128:(ft + 1) * 128],
                                         h1Tb[:, k], start=(k == 0), stop=(k == 3))
                    nc.scalar.activation(gl[:, ft], ps[:], AT.Gelu,
                                         bias=ff1b[:, ft, l:l + 1], scale=1.0)
                h2pre = ap.tile([128, 4, T], f32r, tag="act")
                ff4 = ap.tile([128, 4, T], f32, tag="resc", bufs=2)
                for m in range(4):
                    ps = pmm.tile([128, T], f32, tag="mm")
                    for k in range(16):
                        nc.tensor.matmul(ps[:], wff2[:, k, m * 128:(m + 1) * 128],
                                         gl[:, k], start=(k == 0), stop=(k == 15))
                    nc.scalar.activation(ff4[:, m], ps[:], AT.Identity,
                                         bias=ff2b[:, m, l:l + 1], scale=1.0)
                    nc.vector.tensor_add(h2pre[:, m], ff4[:, m], h1T[:, m].bitcast(f32))

                hT = ap.tile([128, 4, T], f32r, tag="act")
                hTb = ap.tile([128, 4, T], W_DT, tag="actb")
                layernorm(h2pre, 4 * l + 2, hT, hTb)

        # ---- output projection: out[T, IN] = h @ outp_w.T ------------------
        NCH = 9
        CW = IN // NCH            # 1024 columns per chunk
        with tc.tile_pool(name="op_sb", bufs=3) as op:
            hTio = op.tile([128, 4, T], IO_DT, tag="hio", bufs=1)
            nc.vector.tensor_copy(hTio[:], hT[:].bitcast(f32))
            for ncr in range(NCH):
                wc = op.tile([128, 4, CW], IO_DT, tag="wco")
                nc.sync.dma_start(
                    wc[:], outp_wT.rearrange("(kt p) n -> p kt n", p=128)
                    [:, :, ncr * CW:(ncr + 1) * CW])
                for m2 in range(2):
                    for nn in range(2):
                        ps = pmm.tile([128, 512], f32, tag="mm")
                        for k in range(4):
                            nc.tensor.matmul(ps[:], hTio[:, k, m2 * 128:(m2 + 1) * 128],
                                             wc[:, k, nn * 512:(nn + 1) * 512],
                                             start=(k == 0), stop=(k == 3))
                        osb = op.tile([128, 512], f32, tag="osb")
                        nc.vector.tensor_copy(osb[:], ps[:])
                        nc.sync.dma_start(
                            out_d[m2 * 128:(m2 + 1) * 128,
                                  ncr * CW + nn * 512: ncr * CW + (nn + 1) * 512],
                            osb[:])

    _split_multi_waits(nc)
    return nc


# ---------------------------------------------------------------------------
# host wrapper
# ---------------------------------------------------------------------------

def _prepare(inputs, nlayers):
    perm = _perm_src()
    inp_w = np.asarray(inputs["inp_w"], np.float32)
    inp_b = np.asarray(inputs["inp_b"], np.float32)
    in_proj_w = np.asarray(inputs["in_proj_w"], np.float32)
    in_proj_b = np.asarray(inputs["in_proj_b"], np.float32)
    out_w = np.asarray(inputs["out_w"], np.float32)
    out_b = np.asarray(inputs["out_b"], np.float32)
    ln1_w = np.asarray(inputs["ln1_w"], np.float32)
    ln1_b = np.asarray(inputs["ln1_b"], np.float32)
    ln2_w = np.asarray(inputs["ln2_w"], np.float32)
    ln2_b = np.asarray(inputs["ln2_b"], np.float32)
    ff1_w = np.asarray(inputs["ff1_w"], np.float32)
    ff1_b = np.asarray(inputs["ff1_b"], np.float32)
    ff2_w = np.asarray(inputs["ff2_w"], np.float32)
    ff2_b = np.asarray(inputs["ff2_b"], np.float32)
    outp_w = np.asarray(inputs["outp_w"], np.float32)

    common = {}
    common["inp_wT"] = _cast(inp_w[perm, :].T, IO_DT)
    common["outp_wT"] = _cast(outp_w[:, perm].T, IO_DT)
    qkb = np.zeros((D, 2 * nlayers), np.float32)
    obt = np.zeros((D, nlayers), np.float32)
    ff1bt = np.zeros((FF, nlayers), np.float32)
    ff2bt = np.zeros((D, nlayers), np.float32)
    lnc = np.zeros((D, 4 * nlayers), np.float32)
    bvr = np.zeros((nlayers, D), np.float32)
    for l in range(nlayers):
        wq = in_proj_w[l, :D][perm][:, perm] / np.sqrt(HD)
        wk = in_proj_w[l, D:2 * D][perm][:, perm]
        wv = in_proj_w[l, 2 * D:][perm][:, perm]
        common[f"w_qkv_{l}"] = _cast(np.concatenate([wq.T, wk.T, wv.T], axis=1), W_DT)
        common[f"w_out_{l}"] = _cast(out_w[l][perm][:, perm].T, W_DT)
        common[f"w_ff1_{l}"] = _cast(ff1_w[l][:, perm].T, W_DT)
        common[f"w_ff2_{l}"] = _cast(ff2_w[l][perm, :].T, W_DT)
        qkb[:, 2 * l] = in_proj_b[l, :D][perm] / np.sqrt(HD)
        qkb[:, 2 * l + 1] = in_proj_b[l, D:2 * D][perm]
        bvr[l, :] = in_proj_b[l, 2 * D:][perm]
        obt[:, l] = out_b[l][perm]
        ff1bt[:, l] = ff1_b[l]
        ff2bt[:, l] = ff2_b[l][perm]
        lnc[:, 4 * l] = ln1_w[l][perm]
        lnc[:, 4 * l + 1] = ln1_b[l][perm]
        lnc[:, 4 * l + 2] = ln2_w[l][perm]
        lnc[:, 4 * l + 3] = ln2_b[l][perm]
    common["qkb"] = qkb
    common["obt"] = obt
    common["ff1bt"] = ff1bt
    common["ff2bt"] = ff2bt
    common["lnc"] = lnc
    common["bvr"] = _cast(bvr, W_DT)
    common["inpbt"] = inp_b[perm].reshape(D, 1).astype(np.float32)

    # rope tables in permuted space: partition d = h*64 + j
    theta = 1.0 / (10000.0 ** (np.arange(0, HD, 2, dtype=np.float32) / HD))  # [32]
    pos = np.arange(S, dtype=np.float32)
    ang = pos[:, None] * theta[None, :]                  # [S, 32]
    cos_t = np.cos(ang).astype(np.float32)
    sin_t = np.sin(ang).astype(np.float32)
    ct = np.zeros((D, T), np.float32)
    sts = np.zeros((D, T), np.float32)
    s_of_t = np.tile(np.arange(S), BSH)                  # position of each token
    for h in range(NH):
        for j in range(HD // 2):
            ct[h * HD + j, :] = cos_t[s_of_t, j]
            ct[h * HD + HD // 2 + j, :] = cos_t[s_of_t, j]
            sts[h * HD + j, :] = -sin_t[s_of_t, j]
            sts[h * HD + HD // 2 + j, :] = sin_t[s_of_t, j]
    common["ct"] = ct
    common["sts"] = sts

    # additive causal+block mask, transposed (added to scores via maskT.T @ I)
    mask = np.full((128, 128), NEG, np.float32)
    for blk in range(4):
        for i in range(S):
            mask[blk * S + i, blk * S: blk * S + i + 1] = 0.0
    common["maskT"] = _cast(mask.T, W_DT)
    common["idw"] = _cast(np.eye(128, dtype=np.float32), W_DT)
    common["onesr"] = np.ones((128, 128), np.float32)
    common["onesw"] = _cast(np.ones((1, 128), np.float32), W_DT)
    psw = np.zeros((128, 128), np.float32)
    for h2 in range(2):
        b0 = 64 * h2
        for j in range(32):
            psw[b0 + 32 + j, b0 + j] = 1.0      # lhsT[k, m]: out[m] sums in[k]
            psw[b0 + j, b0 + 32 + j] = 1.0
    common["pswap"] = _round_f32r(psw)
    return common


def kernel(**inputs):
    nlayers = _CACHE.setdefault("nlayers", L)
    x = np.asarray(inputs["x"], np.float32)
    if "bass" not in _CACHE:
        _CACHE["bass"] = _build(nlayers)
    nc = _CACHE["bass"]
    common = _prepare(inputs, nlayers)
    in_maps = []
    for c in range(NCORES):
        m = dict(common)
        xc = x[c * BSH:(c + 1) * BSH].reshape(T, IN)
        m["xT"] = _cast(xc.T, IO_DT)
        in_maps.append(m)
    res = run_bass_kernel_spmd(nc, in_maps, core_ids=list(range(NCORES)))
    _CACHE["res"] = res
    outp_b = np.asarray(inputs["outp_b"], np.float32)
    outs = [res.results[c]["out"] + outp_b[None, :] for c in range(NCORES)]
    full = np.concatenate(outs, axis=0).reshape(B, S, LD, Hh, Ww)
    return full.astype(np.float32)

